# revision 5
# baseline (speedup 1.0000x reference)
"""Trainium2 Bass kernel for nn_MimicNetLSTM (2-layer LSTM, H=4096, batch=1, seq=1).

Strategy (tensor-parallel over the 4H gate dim, 8 cores):
  - Core r owns h-indices [512r, 512r+512) of every gate -> 2048 rows of each
    of w_ih0/w_hh0/w_ih1/w_hh1.  Batch-1 matvec chain => HBM-bandwidth bound:
    stream weights once, as few bytes as accuracy allows.
  - Precision: w_ih0/w_hh0/w_ih1 in float8e3 (e3m4), w_hh1 in float16 (its
    input h0[1] is a full-scale randn vector and its output feeds h2 directly,
    so it dominates quantization error; everything else is damped by the
    gate nonlinearities).  All weights host-scaled x128 so e3m4 values sit in
    the normal range; the 1/128 descale is folded into the ACT engine's
    activation(scale=) which is free.  Simulated end-to-end rel err ~4.6e-3.
  - PE matvec with the ACTIVATION (fp16) as the stationary operand (1-column
    LDWEIGHTS) and the weight tiles as the moving operand (4x N=512 per
    128-k-chunk).  Gates land in PSUM partition 0 as [1,2048]=[i|f|g|o].
  - Layer-0 (psum tag g0) and layer-1-whh (psum tag g1) use separate PSUM
    banks (4+4) so whh1 matmuls start with zero PE gap, and the pointwise /
    h1-AllGather hide entirely under the whh1/wih1 weight stream.
  - DMA: 2 MB transfers, alternating between the two HWDGE rings (SP=sync,
    ACT=scalar).  DMA order interleaves whh0 (fp8: PE-slow, DMA-fast) with
    whh1 (fp16: PE-fast, DMA-slow) so PE and DMA rates average out and
    neither stalls.  Small loads go first on the scalar ring; collective
    staging via gpsimd (SWDGE) so it never blocks the weight FIFO.
  - h1 (512 floats/core) is AllGathered between layers (hidden under the
    weight stream).  Heads are per-core partial dot products only: each core
    outputs its 2 partials and the HOST sums 16 floats, adds the bias and
    applies the final sigmoid (the gather/unshard step) -- no device-side
    collective tail.
"""

import os
import numpy as np

import concourse.bass as bass
import concourse.tile as tile
from concourse import bacc, mybir
from concourse.bass_utils import run_bass_kernel_spmd

I, H, L = 512, 4096, 2
NC = 8
SH = H // NC          # 512 h-indices per core
RJ = 4 * SH           # 2048 gate rows per core
FD = mybir.dt.float32
F16 = mybir.dt.float16
F8 = mybir.dt.float8e3

WS = 128.0            # weight prescale (power of 2; e3m4 max 15.5 > 0.109*128)
G8 = 8                # k-chunks per fp8 weight DMA  (2 MB transfers)
G16 = 4               # k-chunks per fp16 weight DMA (2 MB transfers)
G0 = 4                # k-chunks for w_ih0 (single 1 MB transfer)
WBUFS = int(os.environ.get("KERNEL_WBUFS", "4"))

LAST_EXEC_NS = None
LAST_RESULTS = None


def _build_program(iters=1, accum_out=False):
    nc = bacc.Bacc(
        "TRN2",
        target_bir_lowering=False,
        debug=False,
        enable_asserts=False,
        num_devices=NC,
    )

    wih0 = nc.dram_tensor("wih0", [I // G0, G0 * RJ], F8, kind="ExternalInput")
    whh0 = nc.dram_tensor("whh0", [H // G8, G8 * RJ], F8, kind="ExternalInput")
    wih1 = nc.dram_tensor("wih1", [H // G8, G8 * RJ], F8, kind="ExternalInput")
    whh1 = nc.dram_tensor("whh1", [H // G16, G16 * RJ], F16, kind="ExternalInput")
    x_in = nc.dram_tensor("x_in", [128, I // 128], F16, kind="ExternalInput")
    h00 = nc.dram_tensor("h00", [128, H // 128], F16, kind="ExternalInput")
    h01 = nc.dram_tensor("h01", [128, H // 128], F16, kind="ExternalInput")
    c00 = nc.dram_tensor("c00", [1, SH], FD, kind="ExternalInput")
    c01 = nc.dram_tensor("c01", [1, SH], FD, kind="ExternalInput")
    b0 = nc.dram_tensor("b0", [1, RJ], FD, kind="ExternalInput")
    b1 = nc.dram_tensor("b1", [1, RJ], FD, kind="ExternalInput")
    wld = nc.dram_tensor("wld", [1, 2 * SH], FD, kind="ExternalInput")
    out_ld = nc.dram_tensor("out_ld", [1, 2], FD, kind="ExternalOutput")

    SIG = mybir.ActivationFunctionType.Sigmoid
    TANH = mybir.ActivationFunctionType.Tanh

    with tile.TileContext(nc) as tc:
        with (
            tc.tile_pool(name="w", bufs=WBUFS) as wpool,
            tc.tile_pool(name="small", bufs=1) as small,
            tc.tile_pool(name="pw", bufs=1) as pw,
            tc.tile_pool(name="psum", bufs=1, space="PSUM") as ppool,
            tc.tile_pool(name="dram", bufs=1, space="DRAM") as dram,
        ):
          for _it in range(iters):
            # ---- small loads: matmul inputs on the sync(SP) ring ahead of
            # the weights; pointwise inputs on the scalar(ACT) ring, which
            # carries NO weight DMAs so pointwise ACT ops are never stuck
            # behind a weight-DMA issue (head-of-line) ----
            def load_small(name, src, shape, dtype=FD, eng=None):
                t = small.tile(shape, dtype, tag=name)
                (eng or nc.scalar).dma_start(t[:], src[:])
                return t

            x_sb = load_small("x", x_in, [128, I // 128], F16)
            h00_sb = load_small("h00", h00, [128, H // 128], F16)
            h01_sb = load_small("h01", h01, [128, H // 128], F16)
            c00_sb = load_small("c00", c00, [1, SH])
            c01_sb = load_small("c01", c01, [1, SH])
            b0_sb = load_small("b0", b0, [1, RJ])
            b1_sb = load_small("b1", b1, [1, RJ])
            wld_sb = load_small("wld", wld, [1, 2 * SH])

            psum_g0 = ppool.tile([1, RJ], FD, tag="g0")
            psum_g1 = ppool.tile([1, RJ], FD, tag="g1")

            ring = [0]

            def tile_job(wdram, a, G, dtype, rhs_sb, psum, first, last,
                         kchunks):
                """One weight DMA (rows a*128:(a+1)*128 of wdram) + its MMs.

                `first`/`last` mark this matrix as the psum accumulation
                group's opener/closer; kchunks is this matrix's chunk count.
                """
                wt = wpool.tile([128, G * RJ], dtype, tag="w")
                ring[0] += 1
                nc.sync.dma_start(wt[:], wdram[a * 128:(a + 1) * 128, :])
                for d in range(G):
                    c = a * G + d
                    for n in range(4):
                        nc.tensor.matmul(
                            psum[0:1, n * 512:(n + 1) * 512],
                            lhsT=rhs_sb[:, c:c + 1],
                            rhs=wt[:, d * RJ + n * 512:d * RJ + (n + 1) * 512],
                            start=(first and c == 0),
                            stop=(last and c == kchunks - 1),
                        )

            # ---- layer 0 first (close psum g0 ASAP so pointwise0 + the h1
            # AllGather hide under the whh1 stream) ----
            tile_job(wih0, 0, G0, F8, x_sb, psum_g0, True, False, I // 128)
            for a in range(4):
                tile_job(whh0, a, G8, F8, h00_sb, psum_g0, False,
                         a == 3, H // 128)
            # whh1 g1 group opens; all 8 tiles stream while the AG runs
            for j in range(8):
                tile_job(whh1, j, G16, F16, h01_sb, psum_g1,
                         j == 0, False, H // 128)

            # ---- layer-0 pointwise (DVE/ACT; PE keeps streaming whh1) ----
            def pointwise(psum_g, bias_sb, c_sb):
                # [1, 2048] = [i|f|g|o] on partition 0; psum is WS*gates and
                # bias_sb is WS*b, so activation(scale=1/WS) descales for
                # free.  Per-gate slices so ACT overlaps the DVE adds.
                gb = pw.tile([1, RJ], FD, tag="gb")
                act = pw.tile([1, RJ], FD, tag="act")
                for g, fn in ((1, SIG), (0, SIG), (2, TANH), (3, SIG)):
                    s = slice(g * SH, (g + 1) * SH)
                    nc.vector.tensor_add(gb[0:1, s], psum_g[0:1, s],
                                         bias_sb[0:1, s])
                    nc.scalar.activation(act[0:1, s], gb[0:1, s], fn,
                                         scale=1.0 / WS)
                t1 = pw.tile([1, SH], FD, tag="t1")
                nc.vector.tensor_mul(t1[:], act[0:1, SH:2 * SH], c_sb[:])
                t2 = pw.tile([1, SH], FD, tag="t2")
                nc.vector.tensor_mul(
                    t2[:], act[0:1, 0:SH], act[0:1, 2 * SH:3 * SH])
                cn = pw.tile([1, SH], FD, tag="cn")
                nc.vector.tensor_add(cn[:], t1[:], t2[:])
                th = pw.tile([1, SH], FD, tag="th")
                nc.scalar.activation(th[:], cn[:], TANH)
                hn = pw.tile([1, SH], FD, tag="hn")
                nc.vector.tensor_mul(hn[:], act[0:1, 3 * SH:], th[:])
                return hn

            h1_sb = pointwise(psum_g0, b0_sb, c00_sb)

            # AllGather h1 in fp16: 512/core -> 4096 (true h order); staging
            # via gpsimd (SWDGE) so the weight-FIFO rings are never blocked
            h1h_sb = pw.tile([1, SH], F16, tag="h1h")
            nc.vector.tensor_copy(h1h_sb[:], h1_sb[:])
            ag_in = dram.tile([1, SH], F16, tag="ag_in")
            nc.gpsimd.dma_start(ag_in[:], h1h_sb[:])
            ag_out = dram.tile([128, 32], F16, tag="ag_out")
            nc.gpsimd.collective_compute(
                "AllGather",
                mybir.AluOpType.bypass,
                replica_groups=[list(range(NC))],
                ins=[ag_in.opt()],
                outs=[ag_out.opt()],
            )
            h1c_sb = small.tile([128, 32], F16, tag="h1c")
            nc.gpsimd.dma_start(h1c_sb[:], ag_out[:])

            # ---- wih1 tail (fp8; first MM waits on h1c, which lands while
            # the whh1 stream is still draining) ----
            for a in range(4):
                tile_job(wih1, a, G8, F8, h1c_sb, psum_g1, False,
                         a == 3, H // 128)

            h2_sb = pointwise(psum_g1, b1_sb, c01_sb)

            # ---- heads: partial dots over this core's 512 h-indices; host
            # sums the 8 cores' partials (+bias, sigmoid) as the unshard step
            prodl = pw.tile([1, SH], FD, tag="prodl")
            nc.vector.tensor_mul(prodl[:], h2_sb[:], wld_sb[0:1, 0:SH])
            prodd = pw.tile([1, SH], FD, tag="prodd")
            nc.vector.tensor_mul(prodd[:], h2_sb[:], wld_sb[0:1, SH:2 * SH])
            pd_sb = pw.tile([1, 2], FD, tag="pd")
            nc.vector.tensor_reduce(
                pd_sb[0:1, 0:1], prodl[:], mybir.AxisListType.X,
                mybir.AluOpType.add)
            nc.vector.tensor_reduce(
                pd_sb[0:1, 1:2], prodd[:], mybir.AxisListType.X,
                mybir.AluOpType.add)
            if accum_out:
                # timing variants: accumulate so no unrolled body is dead
                # code and the summed output proves every body executed
                nc.gpsimd.dma_start(out_ld[:], pd_sb[:],
                                    accum_op=mybir.AluOpType.add)
            else:
                nc.sync.dma_start(out_ld[:], pd_sb[:])

    nc.compile()
    return nc


def build_program(iters=1, accum_out=None):
    if accum_out is None:
        accum_out = iters > 1
    return _build_program(iters=iters, accum_out=accum_out)


_PROGRAM = None


def _get_program():
    global _PROGRAM
    if _PROGRAM is None:
        _PROGRAM = _build_program()
    return _PROGRAM


def make_in_maps(data, h0, c0, w_ih0, w_hh0, b_ih0, b_hh0,
                 w_ih1, w_hh1, b_ih1, b_hh1, wL, bL, wD, bD):
    """Shard + lay out the full inputs for the 8 cores."""
    import ml_dtypes

    f32 = np.float32
    data, h0, c0 = (np.asarray(a, f32) for a in (data, h0, c0))
    w_ih0, w_hh0, w_ih1, w_hh1 = (
        np.asarray(a, f32) for a in (w_ih0, w_hh0, w_ih1, w_hh1))
    btot0 = WS * (np.asarray(b_ih0, f32) + np.asarray(b_hh0, f32))
    btot1 = WS * (np.asarray(b_ih1, f32) + np.asarray(b_hh1, f32))
    wL, wD = np.asarray(wL, f32), np.asarray(wD, f32)

    p = np.arange(128)
    # contraction slot (c*128 + p) <-> true index, for partition-major rhs
    ordx = (4 * p[None, :] + np.arange(4)[:, None]).reshape(-1)        # I=512
    ordh = (32 * p[None, :] + np.arange(32)[:, None]).reshape(-1)      # H=4096

    x_c = np.ascontiguousarray(data.reshape(128, 4), dtype=np.float16)
    h00_c = np.ascontiguousarray(h0[0, 0].reshape(128, 32), dtype=np.float16)
    h01_c = np.ascontiguousarray(h0[1, 0].reshape(128, 32), dtype=np.float16)

    def regroup(w, G, dtype):
        # [K, RJ] -> [K//G, G*RJ]: one row block = G k-chunks, so a single
        # dma_start moves G contiguous chunks
        Kd = w.shape[0]
        w = (w * WS).astype(np.float32)
        if dtype is np.float16:
            w = w.astype(np.float16)
        else:
            w = np.clip(w, -15.5, 15.5).astype(ml_dtypes.float8_e3m4)
        return np.ascontiguousarray(
            w.reshape(Kd // (128 * G), G, 128, RJ)
            .transpose(0, 2, 1, 3).reshape(Kd // G, G * RJ))

    in_maps = []
    for r in range(NC):
        rows = np.concatenate([g * H + SH * r + np.arange(SH) for g in range(4)])
        sl = slice(SH * r, SH * (r + 1))
        e3 = ml_dtypes.float8_e3m4
        in_maps.append({
            "wih0": regroup(w_ih0[rows].T[ordx], G0, e3),
            "whh0": regroup(w_hh0[rows].T[ordh], G8, e3),
            "wih1": regroup(w_ih1[rows].T[ordh], G8, e3),
            "whh1": regroup(w_hh1[rows].T[ordh], G16, np.float16),
            "x_in": x_c,
            "h00": h00_c,
            "h01": h01_c,
            "c00": np.ascontiguousarray(c0[0, 0, sl].reshape(1, SH)),
            "c01": np.ascontiguousarray(c0[1, 0, sl].reshape(1, SH)),
            "b0": np.ascontiguousarray(btot0[rows].reshape(1, RJ)),
            "b1": np.ascontiguousarray(btot1[rows].reshape(1, RJ)),
            "wld": np.ascontiguousarray(
                np.concatenate([wL[0, sl], wD[0, sl]]).reshape(1, 2 * SH)),
        })
    return in_maps


def kernel(**inputs):
    global LAST_EXEC_NS, LAST_RESULTS
    in_maps = make_in_maps(**inputs)
    nc = _get_program()
    res = run_bass_kernel_spmd(nc, in_maps, core_ids=list(range(NC)))
    LAST_EXEC_NS = res.exec_time_ns
    LAST_RESULTS = res.results
    # host-side unshard: sum the per-core head partials, add bias, sigmoid
    parts = np.stack([np.asarray(r["out_ld"], np.float64).reshape(2)
                      for r in res.results])
    lsum = parts[:, 0].sum() + float(np.asarray(inputs["bL"]).reshape(-1)[0])
    dsum = parts[:, 1].sum() + float(np.asarray(inputs["bD"]).reshape(-1)[0])
    d = np.float32(1.0 / (1.0 + np.exp(-dsum))).reshape(1, 1)
    l = np.float32(lsum).reshape(1, 1)
    return (d, l)
